# revision 43
# baseline (speedup 1.0000x reference)
"""Trainium2 Bass kernel for DifferentiableDLT (batched weighted-DLT homography fit).

Contract: kernel(**inputs) takes FULL inputs
    flow (64, 2, 320, 576) f32, mask (64, 1, 320, 576) f32, img_h, img_w
and returns the FULL output (64, 3, 3) f32.

Design (pure data parallel, 8 batches/core x 8 cores). The 1024 sample points
form a fixed separable 32x32 grid; bilinear sampling touches only a 64-row x
64-column cross product of each field.
  Host (layout only, no arithmetic): shards the batch and gathers exactly the
  needed 64x64 footprint per (image, channel) into FMR [128, 768] bf16 in
  SBUF-ready partition order (flow p=(c,kd,b) free=(sg,a,d,i); mask
  p=(a,kd,b) free=(sg,d,i)), plus three packed constant blobs.
  Device, per core:
    1. Four DMAs split over the two HWDGE rings (S: FMR + small-partition
       consts; A: bf16 interp weights + 128-wide consts); everything lands
       within ~2.5us. Scalar-engine act tables prewarmed meanwhile.
    2. Bilinear interp = one [128,512] DVE multiply against a 4-tap weight
       table (image-coord scale folded in) + two tree-adds -> samples with
       points in the free dim.
    3. PE identity-matmul transpose + grid-offset matmul -> dst image coords
       in PSUM, points on partitions.
    4. Hartley stats via row-sums + ones-matmul broadcast (means) and
       sqrt + ones-matmul (radius); weighted features D = [w, w*cx, w*cy,
       w*r2]; moments C^T @ D on the PE.
    5. Normal equations assembled by a PE matmul against EQG = M0inv @ E
       (M0 = host-constant ideal normal matrix), directly yielding the
       preconditioned system G = I - M0inv*A, c0 = M0inv*b, and the support
       sum. Solved with 4 Horner steps of the Neumann series (spectral
       radius ~0.09); an exact-to-fp32-noise solve at ~1/3 the Gauss-Jordan
       latency.
    6. Denormalize H = inv(T_dst) Hn T_src, sign/scale fix, support gate,
       DMA out (8,3,3).
"""

import math
import numpy as np

import concourse.bass as bass
import concourse.bacc as bacc
import concourse.mybir as mybir
import ml_dtypes
from concourse import tile
from concourse import bass_utils

F32 = mybir.dt.float32
BF16 = mybir.dt.bfloat16
ALU = mybir.AluOpType
ACTF = mybir.ActivationFunctionType

NCORES = 8
BPC = 8          # batches per core
HF, WF = 320, 576
NG = 32          # grid is NG x NG points
NPTS = NG * NG
EPS = 1e-6
KHORNER = 4      # Neumann/Horner applications

# ---------------------------------------------------------------------------
# host-side constant computation
# ---------------------------------------------------------------------------


def _grid_1d(size, n):
    m = int(size * 0.05)
    return np.linspace(m, size - m - 1, n, dtype=np.float32)


class _Consts:
    def __init__(self, img_h, img_w):
        ys = _grid_1d(HF, NG)
        xs = _grid_1d(WF, NG)
        y0 = np.floor(ys).astype(np.int64)
        x0 = np.floor(xs).astype(np.int64)
        wy = (ys - y0).astype(np.float64)
        wx = (xs - x0).astype(np.float64)
        sx = np.float64(np.float32((img_w - 1) / max(WF - 1, 1)))
        sy = np.float64(np.float32((img_h - 1) / max(HF - 1, 1)))
        self.y0 = y0
        self.x0 = x0

        # ---- 4-tap interp weights ----
        wya = np.stack([1 - wy, wy], -1)        # (32, 2) [k, a]
        wxc = np.stack([1 - wx, wx], -1)        # (32, 2) [i, d]
        p = np.arange(128)
        # W4F [p=(c,kd,b), f=(sg,a,d,i)] = wya[4kd+sg, a]*wxc[i, d]*scale(c)
        kd_f = (p % 64) // 8
        c_f = p // 64
        kk = 4 * kd_f[:, None] + np.arange(4)[None, :]          # (128, 4)
        wya_k = wya[kk]                                          # (128, 4, 2)
        sxy_f = np.where(c_f == 0, sx, sy)
        W4F = (wya_k[:, :, :, None, None] * wxc.T[None, None, None, :, :]
               * sxy_f[:, None, None, None, None]).reshape(128, 512)
        # W4M [p=(a,kd,b), f=(sg,d,i)] = wya[4kd+sg, a_m]*wxc[i, d]
        a_m = p // 64
        kd_m = (p % 64) // 8
        kkm = 4 * kd_m[:, None] + np.arange(4)[None, :]
        wya_m = wya[kkm]                                         # (128, 4, 2)
        wya_sel = wya_m[np.arange(128)[:, None], np.arange(4)[None, :], a_m[:, None]]
        W4M = (wya_sel[:, :, None, None] * wxc.T[None, None, :, :]
               ).reshape(128, 256)

        # ---- means route: psPR = RSUM^T @ CBN128 = -flow-mean(c,b) (t-exp);
        # GRIDC[p=(sg,i), f=(c,kd,b)] = grid(c) - gridmean(c) in f32 ----
        j2 = np.arange(128)
        c_j2 = j2 // 64
        b_j2 = j2 % 8
        CBN128 = -(((c_f[:, None] == c_j2[None, :])
                    & ((p % 8)[:, None] == b_j2[None, :]))
                   .astype(np.float32) / NPTS)
        gmx = xs.astype(np.float64).mean() * sx
        gmy = ys.astype(np.float64).mean() * sy
        self.gmx = float(np.float32(gmx))
        self.gmy = float(np.float32(gmy))
        sg_p = p // 32
        i_p = p % 32
        kd_j2 = (j2 % 64) // 8
        gx_p = xs.astype(np.float64)[i_p] * sx - gmx            # (128,) by p
        gy_pf = (ys.astype(np.float64)[(4 * kd_j2[None, :] + sg_p[:, None])
                                       % 32] * sy - gmy)        # (128, 128)
        GRIDC = np.where(c_j2[None, :] == 0, gx_p[:, None], gy_pf
                         ).astype(np.float32)

        # ---- source-point features + T_src immediates ----
        jpt = np.arange(NPTS) // NG
        ipt = np.arange(NPTS) % NG
        gx = xs.astype(np.float64)[ipt]
        gy = ys.astype(np.float64)[jpt]
        sxi = gx * sx
        syi = gy * sy
        mx0, my0 = sxi.mean(), syi.mean()
        cxs, cys = sxi - mx0, syi - my0
        s_src = max(np.sqrt(cxs * cxs + cys * cys).mean() / math.sqrt(2.0), 1e-8)
        u = cxs / s_src
        v = cys / s_src
        self.a_ts = float(np.float32(1.0 / s_src))
        self.c_ts = float(np.float32(-mx0 / s_src))
        self.d_ts = float(np.float32(-my0 / s_src))
        feats = np.stack([u * u, u * v, u, v * v, v, np.ones_like(u)], -1)
        C6 = np.ascontiguousarray(
            feats.reshape(8, 128, 6).transpose(1, 0, 2).reshape(128, 48)
        ).astype(np.float32)

        # ---- E matrices: AUG[r*9+c] = sum_q sum_m E[q][m, r*9+c] * Mq[m] ----
        E = np.zeros((4, 6, 72))
        sym = [[0, 1, 2], [1, 3, 4], [2, 4, 5]]
        for r in range(3):
            for c in range(3):
                m = sym[r][c]
                E[0, m, r * 9 + c] += 1
                E[0, m, (r + 3) * 9 + (c + 3)] += 1
        cr = [[0, 1], [1, 3], [2, 4]]
        for q, r0 in ((1, 0), (2, 3)):
            for r in range(3):
                for c2 in range(2):
                    m = cr[r][c2]
                    E[q, m, (r0 + r) * 9 + 6 + c2] += -1
                    E[q, m, (6 + c2) * 9 + (r0 + r)] += -1
            for r, m in ((0, 2), (1, 4), (2, 5)):
                E[q, m, (r0 + r) * 9 + 8] += 1
        rb = [[0, 1], [1, 3]]
        for r in range(2):
            for c2 in range(2):
                E[3, rb[r][c2], (6 + r) * 9 + 6 + c2] += 1
        E[3, 2, 6 * 9 + 8] += -1
        E[3, 4, 7 * 9 + 8] += -1

        # ---- M0 (ideal normal matrix) -> EQG = M0inv @ E, IME = I-eps*M0inv
        o = np.ones_like(u)
        z = np.zeros_like(u)
        r1 = np.stack([u, v, o, z, z, z, -u * u, -u * v], -1)
        r2 = np.stack([z, z, z, u, v, o, -v * u, -v * v], -1)
        A0 = np.concatenate([r1, r2], 0) * math.sqrt(0.5)
        M0 = A0.T @ A0 + EPS * np.eye(8)
        M0inv = np.linalg.inv(M0)
        EQG = np.einsum('ir,qmrc->qmic', M0inv,
                        E.reshape(4, 6, 8, 9)).reshape(4, 6, 72)
        EQG73 = np.zeros((4, 6, 73))
        EQG73[:, :, 0:72] = EQG
        EQG73[0, 5, 72] = 1.0  # col 72 of q=0 block picks S1 = sum(w)
        EQGP = np.ascontiguousarray(
            EQG73.transpose(1, 0, 2).reshape(6, 292)).astype(np.float32)
        IME = np.tile((np.eye(8) - EPS * M0inv).reshape(1, 64),
                      (8, 1)).astype(np.float32)

        # ---- packed constant blobs ----
        # CALL1 [128, 304] f32: C6 | CBN128 | GRIDC
        IDN = np.eye(128, dtype=np.float32)  # shipped bf16 in CBF
        c1 = np.zeros((128, 304), np.float32)
        c1[:, 0:48] = C6
        c1[:, 48:176] = CBN128
        c1[:, 176:304] = GRIDC
        self.CALL1 = c1
        # CALL2 [8, 356] f32: EQG c0:292 | IME c292:356
        c2b = np.zeros((8, 356), np.float32)
        c2b[0:6, 0:292] = EQGP
        c2b[0:8, 292:356] = IME
        self.CALL2 = c2b
        # CBF [128, 896] bf16: W4F | W4M | IDN
        cb = np.zeros((128, 896), np.float32)
        cb[:, 0:512] = W4F
        cb[:, 512:768] = W4M
        cb[:, 768:896] = IDN
        self.CBF = cb.astype(ml_dtypes.bfloat16)


# ---------------------------------------------------------------------------
# device program
# ---------------------------------------------------------------------------


def _build_program(cc: _Consts):
    nc = bacc.Bacc("TRN2", target_bir_lowering=False, debug=False,
                   num_swdge_queues=1)

    FMR = nc.dram_tensor("FMR", [128, 768], BF16, kind="ExternalInput")
    CALL1 = nc.dram_tensor("CALL1", [128, 304], F32, kind="ExternalInput")
    CALL2 = nc.dram_tensor("CALL2", [8, 356], F32, kind="ExternalInput")
    CBF = nc.dram_tensor("CBF", [128, 896], BF16, kind="ExternalInput")
    Hout = nc.dram_tensor("H", [BPC, 3, 3], F32, kind="ExternalOutput")

    V = nc.vector
    A = nc.scalar
    T = nc.tensor
    S = nc.sync

    with tile.TileContext(nc) as tc:
        with (
            tc.tile_pool(name="sb", bufs=1) as pool,
            tc.tile_pool(name="ps", bufs=1, space="PSUM") as psp,
        ):
            # ---------------- DMAs: critical blobs first on each ring -------
            # flow data (S ring) and flow weights (A ring) land first so the
            # interp multiply can start ASAP; mask halves next; consts last.
            FMR_t = pool.tile([128, 768], BF16, tag="FMR")
            CALL1_t = pool.tile([128, 304], F32, tag="CALL1")
            CALL2_t = pool.tile([8, 356], F32, tag="CALL2")
            CBF_t = pool.tile([128, 896], BF16, tag="CBF")
            A.dma_start(CBF_t[:, 0:512], CBF.ap()[:, 0:512])
            S.dma_start(FMR_t[:, 0:512], FMR.ap()[:, 0:512])
            A.dma_start(CBF_t[:, 512:896], CBF.ap()[:, 512:896])
            S.dma_start(FMR_t[:, 512:768], FMR.ap()[:, 512:768])
            A.dma_start(CALL1_t[:, :], CALL1.ap())
            S.dma_start(CALL2_t[:, :], CALL2.ap())

            C6_t = CALL1_t[:, 0:48]
            CBN_t = CALL1_t[:, 48:176]
            GRIDC_t = CALL1_t[:, 176:304]
            EQG_t = CALL2_t[0:6, 0:292]
            IME_t = CALL2_t[0:8, 292:356]
            W4F_t = CBF_t[:, 0:512]
            W4M_t = CBF_t[:, 512:768]
            IDNB_t = CBF_t[:, 768:896]

            # ---------------- memsets + ACT warmups ----------------
            ONESROW = pool.tile([1, 128], F32)
            V.memset(ONESROW[:, :], 1.0)
            ONESC = pool.tile([128, 1], F32)
            V.memset(ONESC[:, :], 1.0 / (NPTS * math.sqrt(2.0)))
            ONES1 = pool.tile([128, 1], F32)
            V.memset(ONES1[:, :], 1.0)
            IEYE = pool.tile([8, 9], F32)
            V.memset(IEYE[:, :], 0.0)
            V.memset(IEYE[:, 0:9:4], 1.0)
            HN = pool.tile([8, 9], F32)
            V.memset(HN[:, 8:9], 1.0)
            PR = pool.tile([1, 128], F32)
            V.memset(PR[:, :], 0.0)
            ACTJ = pool.tile([8, 2], F32)
            V.memset(ACTJ[:, :], 1.0)
            A.activation(ACTJ[:, 0:1], ACTJ[:, 1:2], ACTF.Sqrt)
            A.activation(ACTJ[:, 0:1], ACTJ[:, 1:2], ACTF.Copy)

            psF = psp.tile([128, 128], F32)
            psM = psp.tile([128, 64], F32)
            psSCW = psp.tile([128, 2], F32)
            psSC = psSCW[:, 0:1]
            D = pool.tile([128, 256], F32)
            Dv = D[:, :].rearrange("p (t q b) -> p t q b", t=8, q=4, b=8)

            # ---------------- flow: interp + transpose ---------------------
            # high priority: keep this chain contiguous at the head of the
            # Vector stream so the mask ops (whose DMA lands later) cannot
            # stall it.
            P = pool.tile([128, 512], BF16)
            Pv = P[:, :].rearrange("p (s a d i) -> p s a d i",
                                   s=4, a=2, d=2, i=32)
            tFv = FMR_t[:, 0:512].rearrange("p (s a d i) -> p s a d i",
                                            s=4, a=2, d=2, i=32)
            W4v = W4F_t.rearrange("p (s a d i) -> p s a d i",
                                  s=4, a=2, d=2, i=32)
            Q = pool.tile([128, 256], BF16)
            Qv = Q[:, :].rearrange("p (s d i) -> p s d i", s=4, d=2, i=32)
            samp = pool.tile([128, 128], BF16)
            sampv = samp[:, :].rearrange("p (s i) -> p s i", s=4, i=32)
            RSUM = pool.tile([128, 1], F32)
            with tc.high_priority():
                # two sg-halves pipelined against the chunked DMAs
                V.tensor_tensor(out=Pv[:, 0:2], in0=tFv[:, 0:2],
                                in1=W4v[:, 0:2], op=ALU.mult)
                V.tensor_tensor(out=Qv[:, 0:2], in0=Pv[:, 0:2, 0, :, :],
                                in1=Pv[:, 0:2, 1, :, :], op=ALU.add)
                V.tensor_tensor(out=sampv[:, 0:2], in0=Qv[:, 0:2, 0, :],
                                in1=Qv[:, 0:2, 1, :], op=ALU.add)
                V.tensor_tensor(out=Pv[:, 2:4], in0=tFv[:, 2:4],
                                in1=W4v[:, 2:4], op=ALU.mult)
                V.tensor_tensor(out=Qv[:, 2:4], in0=Pv[:, 2:4, 0, :, :],
                                in1=Pv[:, 2:4, 1, :, :], op=ALU.add)
                V.tensor_tensor(out=sampv[:, 2:4], in0=Qv[:, 2:4, 0, :],
                                in1=Qv[:, 2:4, 1, :], op=ALU.add)
                V.tensor_reduce(out=RSUM[:, :], in_=samp[:, :].unsqueeze(1),
                                axis=mybir.AxisListType.X, op=ALU.add)
            # psPR[0, f=(c,t,b)] = -flow-mean(c,b); broadcast to all point
            # partitions via a rank-1 ones matmul accumulated into psF.
            T.matmul(psF[0:64, :], samp[:, 0:64], IDNB_t,
                     start=True, stop=False)
            T.matmul(psF[64:128, :], samp[:, 64:128], IDNB_t,
                     start=True, stop=False)
            psPR = psp.tile([1, 128], F32)
            T.matmul(psPR[:, :], RSUM[:, :], CBN_t, start=True, stop=True)
            PRN = pool.tile([1, 128], F32)
            V.tensor_copy(PRN[:, :], psPR[:, :])
            T.matmul(psF[0:64, :], ONESROW[:, 0:64], PRN[:, :],
                     start=False, stop=True)
            T.matmul(psF[64:128, :], ONESROW[:, 0:64], PRN[:, :],
                     start=False, stop=True)

            # ---------------- mask: interp + transpose + relu --------------
            PM = pool.tile([128, 256], BF16)
            V.tensor_tensor(out=PM[:, :], in0=FMR_t[:, 512:768], in1=W4M_t,
                            op=ALU.mult)
            PMhi = pool.tile([64, 256], BF16)
            V.tensor_copy(PMhi[:, :], PM[64:128, :])
            SM = pool.tile([64, 256], BF16)
            V.tensor_tensor(out=SM[:, :], in0=PM[0:64, :], in1=PMhi[:, :],
                            op=ALU.add)
            sampM = pool.tile([64, 128], BF16)
            SMv = SM[:, :].rearrange("p (s d i) -> p s d i", s=4, d=2, i=32)
            smv = sampM[:, :].rearrange("p (s i) -> p s i", s=4, i=32)
            V.tensor_tensor(out=smv, in0=SMv[:, :, 0, :], in1=SMv[:, :, 1, :],
                            op=ALU.add)
            T.matmul(psM[:, :], sampM[:, :], IDNB_t[0:64, 0:64],
                     start=True, stop=True)
            V.tensor_scalar(
                out=Dv[:, :, 0, :],
                in0=psM[:, :].rearrange("p (t b) -> p t b", t=8, b=8),
                scalar1=0.0, op0=ALU.max, scalar2=None)

            # ---------------- radius (CXY = centered dst coords) -----------
            CXY = pool.tile([128, 128], F32)
            V.tensor_tensor(out=CXY[:, :], in0=psF[:, :], in1=GRIDC_t,
                            op=ALU.add)
            SQ = pool.tile([128, 128], F32, tag="SQ")
            V.tensor_tensor(out=SQ[:, :], in0=CXY[:, :], in1=CXY[:, :],
                            op=ALU.mult)
            R2 = pool.tile([128, 64], F32)     # [pl, (t, b)]
            V.tensor_tensor(out=R2[:, :], in0=SQ[:, 0:64], in1=SQ[:, 64:128],
                            op=ALU.add)
            SQR = pool.tile([128, 64], F32)
            A.activation(SQR[:, :], R2[:, :], ACTF.Sqrt)
            psSq = psp.tile([1, 64], F32)
            with tc.high_priority():
                T.matmul(psSq[:, :], ONESC[:, :], SQR[:, :],
                         start=True, stop=True)
            sRow = pool.tile([1, 8], F32)
            V.tensor_reduce(
                out=sRow[:, :],
                in_=psSq[:, :].rearrange("o (t b) -> o b t", t=8, b=8),
                axis=mybir.AxisListType.X, op=ALU.add)
            V.tensor_scalar(out=sRow[:, :], in0=sRow[:, :],
                            scalar1=1e-8, op0=ALU.max, scalar2=None)

            # ---------------- D features + moments ----------------
            V.tensor_tensor(
                out=Dv[:, :, 1:3, :],
                in0=CXY[:, :].rearrange("p (c t b) -> p t c b", c=2, t=8, b=8),
                in1=Dv[:, :, 0:1, :].broadcast_to([128, 8, 2, 8]), op=ALU.mult)
            V.tensor_tensor(
                out=Dv[:, :, 3, :],
                in0=R2[:, :].rearrange("p (t b) -> p t b", t=8, b=8),
                in1=Dv[:, :, 0, :], op=ALU.mult)
            psMom = psp.tile([6, 32], F32)
            for t in range(8):
                T.matmul(psMom[:, :], C6_t[:, 6 * t:6 * t + 6],
                         D[:, 32 * t:32 * t + 32], start=(t == 0), stop=(t == 7))
            Msb = pool.tile([6, 32], F32)
            V.tensor_copy(Msb[:, :], psMom[:, :])

            # ---------------- preconditioned normal equations ----------------
            # q-blocks kept separate; per-batch 1/s, 1/s^2 applied afterwards
            # on partitions (batch = psA partition), off the radius chain.
            psAX = psp.tile([8, 146], F32)
            psA0 = psAX[:, 0:73]
            psA3 = psAX[:, 73:146]
            psA12 = psp.tile([8, 73], F32)
            T.matmul(psA0, Msb[0:6, 0:8], EQG_t[:, 0:73],
                     start=True, stop=True)
            A0sb = pool.tile([8, 73], F32)
            V.tensor_copy(A0sb[:, :], psA0)
            for q in (1, 2):
                T.matmul(psA12[:, :], Msb[0:6, 8 * q:8 * q + 8],
                         EQG_t[:, 73 * q:73 * q + 73], start=(q == 1),
                         stop=(q == 2))
            T.matmul(psA3, Msb[0:6, 24:32], EQG_t[:, 219:292],
                     start=True, stop=True)

            # ---------------- per-batch scalars to partitions --------------
            V.tensor_scalar(out=PR[:, 0:8], in0=psPR[:, 0:8], scalar1=-1.0,
                            op0=ALU.mult, scalar2=cc.gmx, op1=ALU.add)
            V.tensor_scalar(out=PR[:, 32:40], in0=psPR[:, 64:72], scalar1=-1.0,
                            op0=ALU.mult, scalar2=cc.gmy, op1=ALU.add)
            A.activation(PR[:, 64:72], sRow[:, :], ACTF.Copy)
            T.transpose(psSC[:, :], PR[:, :], ONES1[0:1, 0:1])
            SCC = pool.tile([8, 4], F32)
            A.activation(SCC[:, 0:1], psSC[0:8, :], ACTF.Copy)      # mx
            A.activation(SCC[:, 1:2], psSC[32:40, :], ACTF.Copy)    # my
            A.activation(SCC[:, 2:3], psSC[64:72, :], ACTF.Copy)    # s_dst

            # ---------------- Horner / Neumann solve ----------------
            IR8T = pool.tile([8, 2], F32)
            V.reciprocal(IR8T[:, 0:1], SCC[:, 2:3])
            V.tensor_tensor(out=IR8T[:, 1:2], in0=IR8T[:, 0:1],
                            in1=IR8T[:, 0:1], op=ALU.mult)
            U2 = pool.tile([8, 73], F32)
            V.scalar_tensor_tensor(out=U2[:, :], in0=psA12[:, :],
                                   scalar=IR8T[:, 0:1], in1=A0sb[:, :],
                                   op0=ALU.mult, op1=ALU.add)
            V.scalar_tensor_tensor(out=U2[:, :], in0=psA3,
                                   scalar=IR8T[:, 1:2], in1=U2[:, :],
                                   op0=ALU.mult, op1=ALU.add)
            GT = pool.tile([8, 64], F32)
            V.tensor_tensor(
                out=GT[:, :].rearrange("p (i j) -> p i j", i=8, j=8),
                in0=IME_t[:, :].rearrange("p (i j) -> p i j", i=8, j=8),
                in1=U2[:, 0:72].rearrange("p (i k) -> p i k", i=8, k=9)[:, :, 0:8],
                op=ALU.subtract)
            C0 = pool.tile([8, 8], F32)
            V.tensor_copy(C0[:, :], U2[:, 8:72:9])
            Gv3 = GT[:, :].rearrange("p (i j) -> p i j", i=8, j=8)
            PH = pool.tile([8, 64], F32)
            PHv = PH[:, :].rearrange("p (i j) -> p i j", i=8, j=8)
            YT = pool.tile([8, 8], F32)
            XC = pool.tile([8, 8], F32)
            for it in range(KHORNER):
                xin = C0 if it == 0 else XC
                V.tensor_tensor(out=PHv, in0=Gv3,
                                in1=xin[:, :].unsqueeze(1).broadcast_to([8, 8, 8]),
                                op=ALU.mult)
                V.tensor_reduce(out=YT[:, :], in_=PHv,
                                axis=mybir.AxisListType.X, op=ALU.add)
                xout = HN[:, 0:8] if it == KHORNER - 1 else XC[:, :]
                V.tensor_tensor(out=xout, in0=YT[:, :], in1=C0[:, :], op=ALU.add)

            # ---------------- denormalize + gate ----------------
            V.tensor_scalar(out=SCC[:, 3:4], in0=U2[:, 72:73],
                            scalar1=NPTS * 1e-4, op0=ALU.is_gt, scalar2=None)
            IG = pool.tile([8, 1], F32)
            V.tensor_scalar(out=IG[:, :], in0=SCC[:, 3:4], scalar1=-1.0,
                            op0=ALU.mult, scalar2=1.0, op1=ALU.add)
            TI = pool.tile([8, 9], F32)
            V.tensor_scalar(out=TI[:, :], in0=IEYE[:, :], scalar1=IG[:, :],
                            op0=ALU.mult, scalar2=None)
            mx_sc, my_sc = SCC[:, 0:1], SCC[:, 1:2]
            s_sc, g_sc = SCC[:, 2:3], SCC[:, 3:4]
            H2 = pool.tile([8, 9], F32)
            # H2[2,2] = c_ts*h6 + d_ts*h7 + 1 (early, feeds the sign chain)
            W1 = pool.tile([8, 1], F32)
            V.tensor_scalar(out=W1[:, :], in0=HN[:, 6:7], scalar1=cc.c_ts,
                            op0=ALU.mult, scalar2=1.0, op1=ALU.add)
            V.scalar_tensor_tensor(out=H2[:, 8:9], in0=HN[:, 7:8], scalar=cc.d_ts,
                                   in1=W1[:, :], op0=ALU.mult, op1=ALU.add)
            ABSD = pool.tile([8, 1], F32)
            NEGH = pool.tile([8, 1], F32)
            V.tensor_scalar(out=NEGH[:, :], in0=H2[:, 8:9], scalar1=-1.0,
                            op0=ALU.mult, scalar2=None)
            V.tensor_tensor(out=ABSD[:, :], in0=H2[:, 8:9], in1=NEGH[:, :],
                            op=ALU.max)
            SGN = pool.tile([8, 1], F32)
            V.tensor_scalar(out=SGN[:, :], in0=H2[:, 8:9], scalar1=0.0,
                            op0=ALU.is_lt, scalar2=-2.0, op1=ALU.mult)
            V.tensor_scalar(out=SGN[:, :], in0=SGN[:, :], scalar1=1.0,
                            op0=ALU.add, scalar2=None)
            DEN = pool.tile([8, 1], F32)
            V.tensor_scalar(out=DEN[:, :], in0=ABSD[:, :], scalar1=1e-8,
                            op0=ALU.max, scalar2=SGN[:, :], op1=ALU.mult)
            RECD = pool.tile([8, 1], F32)
            V.reciprocal(RECD[:, :], DEN[:, :])
            RG = pool.tile([8, 1], F32)
            V.tensor_tensor(out=RG[:, :], in0=RECD[:, :], in1=g_sc, op=ALU.mult)
            # rows of inv(T_dst) @ Hn
            T1 = pool.tile([8, 6], F32)
            H1 = pool.tile([8, 9], F32)
            V.tensor_scalar(out=T1[:, :], in0=HN[:, 0:6], scalar1=s_sc,
                            op0=ALU.mult, scalar2=None)
            V.scalar_tensor_tensor(out=H1[:, 0:3], in0=HN[:, 6:9], scalar=mx_sc,
                                   in1=T1[:, 0:3], op0=ALU.mult, op1=ALU.add)
            V.scalar_tensor_tensor(out=H1[:, 3:6], in0=HN[:, 6:9], scalar=my_sc,
                                   in1=T1[:, 3:6], op0=ALU.mult, op1=ALU.add)
            V.tensor_copy(H1[:, 6:9], HN[:, 6:9])
            # columns: @ T_src
            H1v = H1[:, :].rearrange("p (r c) -> p r c", r=3, c=3)
            H2v = H2[:, :].rearrange("p (r c) -> p r c", r=3, c=3)
            V.tensor_scalar(out=H2v[:, :, 0:2], in0=H1v[:, :, 0:2],
                            scalar1=cc.a_ts, op0=ALU.mult, scalar2=None)
            T2 = pool.tile([8, 2], F32)
            T3 = pool.tile([8, 2], F32)
            V.tensor_scalar(out=T2[:, :], in0=H1[:, 0:4:3], scalar1=cc.c_ts,
                            op0=ALU.mult, scalar2=None)
            V.scalar_tensor_tensor(out=T3[:, :], in0=H1[:, 1:5:3], scalar=cc.d_ts,
                                   in1=T2[:, :], op0=ALU.mult, op1=ALU.add)
            V.tensor_tensor(out=H2[:, 2:6:3], in0=T3[:, :], in1=H1[:, 2:6:3],
                            op=ALU.add)
            OUTt = pool.tile([8, 9], F32)
            V.scalar_tensor_tensor(out=OUTt[:, :], in0=H2[:, :], scalar=RG[:, :],
                                   in1=TI[:, :], op0=ALU.mult, op1=ALU.add)
            S.dma_start(Hout.ap().rearrange("b r c -> b (r c)"), OUTt[:, :])

    nc.compile()
    return nc


# ---------------------------------------------------------------------------
# host wrapper
# ---------------------------------------------------------------------------

_CACHE = {}


def _get(img_h, img_w):
    key = (int(img_h), int(img_w))
    if key not in _CACHE:
        cc = _Consts(*key)
        _CACHE[key] = (cc, _build_program(cc))
    return _CACHE[key]


def _in_maps(cc, flow, mask):
    flow = np.ascontiguousarray(flow, np.float32)
    mask = np.ascontiguousarray(mask, np.float32)
    rows2 = np.stack([cc.y0, cc.y0 + 1], -1).reshape(-1)   # (64,) = (k, a)
    cols2 = np.stack([cc.x0, cc.x0 + 1], -1).reshape(-1)   # (64,) = (i, d)
    maps = []
    for cix in range(NCORES):
        fl = flow[cix * BPC:(cix + 1) * BPC]          # (8, 2, H, W)
        mk = mask[cix * BPC:(cix + 1) * BPC]          # (8, 1, H, W)
        # FR [p=(c,kd,b), f=(sg,a,d,i)]
        fr = fl[:, :, rows2][:, :, :, cols2]          # (8, 2, 64, 64)
        fr = fr.reshape(BPC, 2, 8, 4, 2, 32, 2)       # b c kd sg a i d
        FR = fr.transpose(1, 2, 0, 3, 4, 6, 5).reshape(128, 512)
        # MR [p=(a,kd,b), f=(sg,d,i)]
        mr = mk[:, 0][:, rows2][:, :, cols2]          # (8, 64, 64)
        mr = mr.reshape(BPC, 8, 4, 2, 32, 2)          # b kd sg a i d
        MR = mr.transpose(3, 1, 0, 2, 5, 4).reshape(128, 256)
        fmr = np.ascontiguousarray(
            np.concatenate([FR, MR], axis=1).astype(ml_dtypes.bfloat16))
        maps.append({"FMR": fmr, "CALL1": cc.CALL1, "CALL2": cc.CALL2,
                     "CBF": cc.CBF})
    return maps


def run(flow, mask, img_h, img_w, trace=False, **spmd_kwargs):
    cc, nc = _get(img_h, img_w)
    res = bass_utils.run_bass_kernel_spmd(
        nc, _in_maps(cc, flow, mask), list(range(NCORES)), trace=trace, **spmd_kwargs
    )
    out = np.concatenate([res.results[c]["H"] for c in range(NCORES)], axis=0)
    return out.astype(np.float32), res


def kernel(flow, mask, img_h, img_w):
    out, _ = run(flow, mask, img_h, img_w)
    return out


# revision 44
# speedup vs baseline: 1.0290x; 1.0290x over previous
"""Trainium2 Bass kernel for DifferentiableDLT (batched weighted-DLT homography fit).

Contract: kernel(**inputs) takes FULL inputs
    flow (64, 2, 320, 576) f32, mask (64, 1, 320, 576) f32, img_h, img_w
and returns the FULL output (64, 3, 3) f32.

Design (pure data parallel, 8 batches/core x 8 cores). The 1024 sample points
form a fixed separable 32x32 grid; bilinear sampling touches only a 64-row x
64-column cross product of each field.
  Host (layout only, no arithmetic): shards the batch and gathers exactly the
  needed 64x64 footprint per (image, channel) into FMR [128, 768] bf16 in
  SBUF-ready partition order (flow p=(c,kd,b) free=(sg,a,d,i); mask
  p=(a,kd,b) free=(sg,d,i)), plus three packed constant blobs.
  Device, per core:
    1. Four DMAs split over the two HWDGE rings (S: FMR + small-partition
       consts; A: bf16 interp weights + 128-wide consts); everything lands
       within ~2.5us. Scalar-engine act tables prewarmed meanwhile.
    2. Bilinear interp = one [128,512] DVE multiply against a 4-tap weight
       table (image-coord scale folded in) + two tree-adds -> samples with
       points in the free dim.
    3. PE identity-matmul transpose + grid-offset matmul -> dst image coords
       in PSUM, points on partitions.
    4. Hartley stats via row-sums + ones-matmul broadcast (means) and
       sqrt + ones-matmul (radius); weighted features D = [w, w*cx, w*cy,
       w*r2]; moments C^T @ D on the PE.
    5. Normal equations assembled by a PE matmul against EQG = M0inv @ E
       (M0 = host-constant ideal normal matrix), directly yielding the
       preconditioned system G = I - M0inv*A, c0 = M0inv*b, and the support
       sum. Solved with 4 Horner steps of the Neumann series (spectral
       radius ~0.09); an exact-to-fp32-noise solve at ~1/3 the Gauss-Jordan
       latency.
    6. Denormalize H = inv(T_dst) Hn T_src, sign/scale fix, support gate,
       DMA out (8,3,3).
"""

import math
import numpy as np

import concourse.bass as bass
import concourse.bacc as bacc
import concourse.mybir as mybir
import ml_dtypes
from concourse import tile
from concourse import bass_utils

F32 = mybir.dt.float32
BF16 = mybir.dt.bfloat16
ALU = mybir.AluOpType
ACTF = mybir.ActivationFunctionType

NCORES = 8
BPC = 8          # batches per core
HF, WF = 320, 576
NG = 32          # grid is NG x NG points
NPTS = NG * NG
EPS = 1e-6
KHORNER = 4      # Neumann/Horner applications

# ---------------------------------------------------------------------------
# host-side constant computation
# ---------------------------------------------------------------------------


def _grid_1d(size, n):
    m = int(size * 0.05)
    return np.linspace(m, size - m - 1, n, dtype=np.float32)


class _Consts:
    def __init__(self, img_h, img_w):
        ys = _grid_1d(HF, NG)
        xs = _grid_1d(WF, NG)
        y0 = np.floor(ys).astype(np.int64)
        x0 = np.floor(xs).astype(np.int64)
        wy = (ys - y0).astype(np.float64)
        wx = (xs - x0).astype(np.float64)
        sx = np.float64(np.float32((img_w - 1) / max(WF - 1, 1)))
        sy = np.float64(np.float32((img_h - 1) / max(HF - 1, 1)))
        self.y0 = y0
        self.x0 = x0

        # ---- 4-tap interp weights ----
        wya = np.stack([1 - wy, wy], -1)        # (32, 2) [k, a]
        wxc = np.stack([1 - wx, wx], -1)        # (32, 2) [i, d]
        p = np.arange(128)
        # W4F [p=(c,kd,b), f=(sg,a,d,i)] = wya[4kd+sg, a]*wxc[i, d]*scale(c)
        kd_f = (p % 64) // 8
        c_f = p // 64
        kk = 4 * kd_f[:, None] + np.arange(4)[None, :]          # (128, 4)
        wya_k = wya[kk]                                          # (128, 4, 2)
        sxy_f = np.where(c_f == 0, sx, sy)
        W4F = (wya_k[:, :, :, None, None] * wxc.T[None, None, None, :, :]
               * sxy_f[:, None, None, None, None]).reshape(128, 512)
        # W4M [p=(a,kd,b), f=(sg,d,i)] = wya[4kd+sg, a_m]*wxc[i, d]
        a_m = p // 64
        kd_m = (p % 64) // 8
        kkm = 4 * kd_m[:, None] + np.arange(4)[None, :]
        wya_m = wya[kkm]                                         # (128, 4, 2)
        wya_sel = wya_m[np.arange(128)[:, None], np.arange(4)[None, :], a_m[:, None]]
        W4M = (wya_sel[:, :, None, None] * wxc.T[None, None, :, :]
               ).reshape(128, 256)

        # ---- means route: psPR = RSUM^T @ CBN128 = -flow-mean(c,b) (t-exp);
        # GRIDC[p=(sg,i), f=(c,kd,b)] = grid(c) - gridmean(c) in f32 ----
        j2 = np.arange(128)
        c_j2 = j2 // 64
        b_j2 = j2 % 8
        CBN128 = -(((c_f[:, None] == c_j2[None, :])
                    & ((p % 8)[:, None] == b_j2[None, :]))
                   .astype(np.float32) / NPTS)
        gmx = xs.astype(np.float64).mean() * sx
        gmy = ys.astype(np.float64).mean() * sy
        self.gmx = float(np.float32(gmx))
        self.gmy = float(np.float32(gmy))
        sg_p = p // 32
        i_p = p % 32
        kd_j2 = (j2 % 64) // 8
        gx_p = xs.astype(np.float64)[i_p] * sx - gmx            # (128,) by p
        gy_pf = (ys.astype(np.float64)[(4 * kd_j2[None, :] + sg_p[:, None])
                                       % 32] * sy - gmy)        # (128, 128)
        GRIDC = np.where(c_j2[None, :] == 0, gx_p[:, None], gy_pf
                         ).astype(np.float32)

        # ---- source-point features + T_src immediates ----
        jpt = np.arange(NPTS) // NG
        ipt = np.arange(NPTS) % NG
        gx = xs.astype(np.float64)[ipt]
        gy = ys.astype(np.float64)[jpt]
        sxi = gx * sx
        syi = gy * sy
        mx0, my0 = sxi.mean(), syi.mean()
        cxs, cys = sxi - mx0, syi - my0
        s_src = max(np.sqrt(cxs * cxs + cys * cys).mean() / math.sqrt(2.0), 1e-8)
        u = cxs / s_src
        v = cys / s_src
        self.a_ts = float(np.float32(1.0 / s_src))
        self.c_ts = float(np.float32(-mx0 / s_src))
        self.d_ts = float(np.float32(-my0 / s_src))
        feats = np.stack([u * u, u * v, u, v * v, v, np.ones_like(u)], -1)
        C6 = np.ascontiguousarray(
            feats.reshape(8, 128, 6).transpose(1, 0, 2).reshape(128, 48)
        ).astype(np.float32)

        # ---- E matrices: AUG[r*9+c] = sum_q sum_m E[q][m, r*9+c] * Mq[m] ----
        E = np.zeros((4, 6, 72))
        sym = [[0, 1, 2], [1, 3, 4], [2, 4, 5]]
        for r in range(3):
            for c in range(3):
                m = sym[r][c]
                E[0, m, r * 9 + c] += 1
                E[0, m, (r + 3) * 9 + (c + 3)] += 1
        cr = [[0, 1], [1, 3], [2, 4]]
        for q, r0 in ((1, 0), (2, 3)):
            for r in range(3):
                for c2 in range(2):
                    m = cr[r][c2]
                    E[q, m, (r0 + r) * 9 + 6 + c2] += -1
                    E[q, m, (6 + c2) * 9 + (r0 + r)] += -1
            for r, m in ((0, 2), (1, 4), (2, 5)):
                E[q, m, (r0 + r) * 9 + 8] += 1
        rb = [[0, 1], [1, 3]]
        for r in range(2):
            for c2 in range(2):
                E[3, rb[r][c2], (6 + r) * 9 + 6 + c2] += 1
        E[3, 2, 6 * 9 + 8] += -1
        E[3, 4, 7 * 9 + 8] += -1

        # ---- M0 (ideal normal matrix) -> EQG = M0inv @ E, IME = I-eps*M0inv
        o = np.ones_like(u)
        z = np.zeros_like(u)
        r1 = np.stack([u, v, o, z, z, z, -u * u, -u * v], -1)
        r2 = np.stack([z, z, z, u, v, o, -v * u, -v * v], -1)
        A0 = np.concatenate([r1, r2], 0) * math.sqrt(0.5)
        M0 = A0.T @ A0 + EPS * np.eye(8)
        M0inv = np.linalg.inv(M0)
        EQG = np.einsum('ir,qmrc->qmic', M0inv,
                        E.reshape(4, 6, 8, 9)).reshape(4, 6, 72)
        EQG73 = np.zeros((4, 6, 73))
        EQG73[:, :, 0:72] = EQG
        EQG73[0, 5, 72] = 1.0  # col 72 of q=0 block picks S1 = sum(w)
        EQGP = np.ascontiguousarray(
            EQG73.transpose(1, 0, 2).reshape(6, 292)).astype(np.float32)
        IME = np.tile((np.eye(8) - EPS * M0inv).reshape(1, 64),
                      (8, 1)).astype(np.float32)

        # ---- packed constant blobs ----
        # CALL1 [128, 304] f32: C6 | CBN128 | GRIDC
        IDN = np.eye(128, dtype=np.float32)  # shipped bf16 in CBF
        c1 = np.zeros((128, 304), np.float32)
        c1[:, 0:48] = C6
        c1[:, 48:176] = CBN128
        c1[:, 176:304] = GRIDC
        self.CALL1 = c1
        # CALL2 [8, 356] f32: EQG c0:292 | IME c292:356
        c2b = np.zeros((8, 356), np.float32)
        c2b[0:6, 0:292] = EQGP
        c2b[0:8, 292:356] = IME
        self.CALL2 = c2b
        # CBF [128, 896] bf16: W4F | W4M | IDN
        cb = np.zeros((128, 896), np.float32)
        cb[:, 0:512] = W4F
        cb[:, 512:768] = W4M
        cb[:, 768:896] = IDN
        self.CBF = cb.astype(ml_dtypes.bfloat16)


# ---------------------------------------------------------------------------
# device program
# ---------------------------------------------------------------------------


def _build_program(cc: _Consts):
    nc = bacc.Bacc("TRN2", target_bir_lowering=False, debug=False,
                   num_swdge_queues=1)

    FMR = nc.dram_tensor("FMR", [128, 768], BF16, kind="ExternalInput")
    CALL1 = nc.dram_tensor("CALL1", [128, 304], F32, kind="ExternalInput")
    CALL2 = nc.dram_tensor("CALL2", [8, 356], F32, kind="ExternalInput")
    CBF = nc.dram_tensor("CBF", [128, 896], BF16, kind="ExternalInput")
    Hout = nc.dram_tensor("H", [BPC, 3, 3], F32, kind="ExternalOutput")

    V = nc.vector
    A = nc.scalar
    T = nc.tensor
    S = nc.sync

    with tile.TileContext(nc) as tc:
        with (
            tc.tile_pool(name="sb", bufs=1) as pool,
            tc.tile_pool(name="ps", bufs=1, space="PSUM") as psp,
        ):
            # ---------------- DMAs: critical blobs first on each ring -------
            # flow data (S ring) and flow weights (A ring) land first so the
            # interp multiply can start ASAP; mask halves next; consts last.
            FMR_t = pool.tile([128, 768], BF16, tag="FMR")
            CALL1_t = pool.tile([128, 304], F32, tag="CALL1")
            CALL2_t = pool.tile([8, 356], F32, tag="CALL2")
            CBF_t = pool.tile([128, 896], BF16, tag="CBF")
            A.dma_start(CBF_t[:, 0:512], CBF.ap()[:, 0:512])
            S.dma_start(FMR_t[:, 0:512], FMR.ap()[:, 0:512])
            A.dma_start(CBF_t[:, 512:896], CBF.ap()[:, 512:896])
            S.dma_start(FMR_t[:, 512:768], FMR.ap()[:, 512:768])
            A.dma_start(CALL1_t[:, :], CALL1.ap())
            S.dma_start(CALL2_t[:, :], CALL2.ap())

            C6_t = CALL1_t[:, 0:48]
            CBN_t = CALL1_t[:, 48:176]
            GRIDC_t = CALL1_t[:, 176:304]
            EQG_t = CALL2_t[0:6, 0:292]
            IME_t = CALL2_t[0:8, 292:356]
            W4F_t = CBF_t[:, 0:512]
            W4M_t = CBF_t[:, 512:768]
            IDNB_t = CBF_t[:, 768:896]

            # ---------------- memsets + ACT warmups ----------------
            ONESROW = pool.tile([1, 128], F32)
            V.memset(ONESROW[:, :], 1.0)
            ONESC = pool.tile([128, 1], F32)
            V.memset(ONESC[:, :], 1.0 / (NPTS * math.sqrt(2.0)))
            ONES1 = pool.tile([128, 1], F32)
            V.memset(ONES1[:, :], 1.0)
            IEYE = pool.tile([8, 9], F32)
            V.memset(IEYE[:, :], 0.0)
            V.memset(IEYE[:, 0:9:4], 1.0)
            HN = pool.tile([8, 9], F32)
            V.memset(HN[:, 8:9], 1.0)
            PR = pool.tile([1, 128], F32)
            V.memset(PR[:, :], 0.0)
            ACTJ = pool.tile([8, 2], F32)
            V.memset(ACTJ[:, :], 1.0)
            A.activation(ACTJ[:, 0:1], ACTJ[:, 1:2], ACTF.Sqrt)
            A.activation(ACTJ[:, 0:1], ACTJ[:, 1:2], ACTF.Copy)

            psF = psp.tile([128, 128], F32)
            psM = psp.tile([128, 64], F32)
            psSCW = psp.tile([128, 2], F32)
            psSC = psSCW[:, 0:1]
            D = pool.tile([128, 256], F32)
            Dv = D[:, :].rearrange("p (t q b) -> p t q b", t=8, q=4, b=8)

            # ---------------- flow: interp + transpose ---------------------
            # high priority: keep this chain contiguous at the head of the
            # Vector stream so the mask ops (whose DMA lands later) cannot
            # stall it.
            P = pool.tile([128, 512], BF16)
            Pv = P[:, :].rearrange("p (s a d i) -> p s a d i",
                                   s=4, a=2, d=2, i=32)
            tFv = FMR_t[:, 0:512].rearrange("p (s a d i) -> p s a d i",
                                            s=4, a=2, d=2, i=32)
            W4v = W4F_t.rearrange("p (s a d i) -> p s a d i",
                                  s=4, a=2, d=2, i=32)
            Q = pool.tile([128, 256], BF16)
            Qv = Q[:, :].rearrange("p (s d i) -> p s d i", s=4, d=2, i=32)
            samp = pool.tile([128, 128], BF16)
            sampv = samp[:, :].rearrange("p (s i) -> p s i", s=4, i=32)
            RSUM = pool.tile([128, 1], F32)
            with tc.high_priority():
                # two sg-halves pipelined against the chunked DMAs
                V.tensor_tensor(out=Pv[:, 0:2], in0=tFv[:, 0:2],
                                in1=W4v[:, 0:2], op=ALU.mult)
                V.tensor_tensor(out=Qv[:, 0:2], in0=Pv[:, 0:2, 0, :, :],
                                in1=Pv[:, 0:2, 1, :, :], op=ALU.add)
                V.tensor_tensor(out=sampv[:, 0:2], in0=Qv[:, 0:2, 0, :],
                                in1=Qv[:, 0:2, 1, :], op=ALU.add)
                V.tensor_tensor(out=Pv[:, 2:4], in0=tFv[:, 2:4],
                                in1=W4v[:, 2:4], op=ALU.mult)
                V.tensor_tensor(out=Qv[:, 2:4], in0=Pv[:, 2:4, 0, :, :],
                                in1=Pv[:, 2:4, 1, :, :], op=ALU.add)
                V.tensor_tensor(out=sampv[:, 2:4], in0=Qv[:, 2:4, 0, :],
                                in1=Qv[:, 2:4, 1, :], op=ALU.add)
                V.tensor_reduce(out=RSUM[:, :], in_=samp[:, :].unsqueeze(1),
                                axis=mybir.AxisListType.X, op=ALU.add)
            # psPR[0, f=(c,t,b)] = -flow-mean(c,b); broadcast to all point
            # partitions via a rank-1 ones matmul accumulated into psF.
            T.matmul(psF[0:64, :], samp[:, 0:64], IDNB_t,
                     start=True, stop=False)
            T.matmul(psF[64:128, :], samp[:, 64:128], IDNB_t,
                     start=True, stop=False)
            psPR = psp.tile([1, 128], F32)
            T.matmul(psPR[:, :], RSUM[:, :], CBN_t, start=True, stop=True)
            PRN = pool.tile([1, 128], F32)
            V.tensor_copy(PRN[:, :], psPR[:, :])
            T.matmul(psF[0:64, :], ONESROW[:, 0:64], PRN[:, :],
                     start=False, stop=True)
            T.matmul(psF[64:128, :], ONESROW[:, 0:64], PRN[:, :],
                     start=False, stop=True)

            # ---------------- mask: interp + transpose + relu --------------
            PM = pool.tile([128, 256], BF16)
            V.tensor_tensor(out=PM[:, :], in0=FMR_t[:, 512:768], in1=W4M_t,
                            op=ALU.mult)
            PMhi = pool.tile([64, 256], BF16)
            V.tensor_copy(PMhi[:, :], PM[64:128, :])
            SM = pool.tile([64, 256], BF16)
            V.tensor_tensor(out=SM[:, :], in0=PM[0:64, :], in1=PMhi[:, :],
                            op=ALU.add)
            sampM = pool.tile([64, 128], BF16)
            SMv = SM[:, :].rearrange("p (s d i) -> p s d i", s=4, d=2, i=32)
            smv = sampM[:, :].rearrange("p (s i) -> p s i", s=4, i=32)
            V.tensor_tensor(out=smv, in0=SMv[:, :, 0, :], in1=SMv[:, :, 1, :],
                            op=ALU.add)
            T.matmul(psM[:, :], sampM[:, :], IDNB_t[0:64, 0:64],
                     start=True, stop=True)
            V.tensor_scalar(
                out=Dv[:, :, 0, :],
                in0=psM[:, :].rearrange("p (t b) -> p t b", t=8, b=8),
                scalar1=0.0, op0=ALU.max, scalar2=None)

            # ---------------- radius (CXY = centered dst coords) -----------
            CXY = pool.tile([128, 128], F32)
            V.tensor_tensor(out=CXY[:, :], in0=psF[:, :], in1=GRIDC_t,
                            op=ALU.add)
            SQ = pool.tile([128, 128], F32, tag="SQ")
            V.tensor_tensor(out=SQ[:, :], in0=CXY[:, :], in1=CXY[:, :],
                            op=ALU.mult)
            R2 = pool.tile([128, 64], F32)     # [pl, (t, b)]
            V.tensor_tensor(out=R2[:, :], in0=SQ[:, 0:64], in1=SQ[:, 64:128],
                            op=ALU.add)
            SQR = pool.tile([128, 64], F32)
            A.activation(SQR[:, :], R2[:, :], ACTF.Sqrt)
            psSq = psp.tile([1, 64], F32)
            with tc.high_priority():
                T.matmul(psSq[:, :], ONESC[:, :], SQR[:, :],
                         start=True, stop=True)
            sRow = pool.tile([1, 8], F32)
            V.tensor_reduce(
                out=sRow[:, :],
                in_=psSq[:, :].rearrange("o (t b) -> o b t", t=8, b=8),
                axis=mybir.AxisListType.X, op=ALU.add)
            V.tensor_scalar(out=sRow[:, :], in0=sRow[:, :],
                            scalar1=1e-8, op0=ALU.max, scalar2=None)

            # ---------------- D features + moments ----------------
            V.tensor_tensor(
                out=Dv[:, :, 1:3, :],
                in0=CXY[:, :].rearrange("p (c t b) -> p t c b", c=2, t=8, b=8),
                in1=Dv[:, :, 0:1, :].broadcast_to([128, 8, 2, 8]), op=ALU.mult)
            V.tensor_tensor(
                out=Dv[:, :, 3, :],
                in0=R2[:, :].rearrange("p (t b) -> p t b", t=8, b=8),
                in1=Dv[:, :, 0, :], op=ALU.mult)
            psMom = psp.tile([6, 32], F32)
            for t in range(8):
                T.matmul(psMom[:, :], C6_t[:, 6 * t:6 * t + 6],
                         D[:, 32 * t:32 * t + 32], start=(t == 0), stop=(t == 7))
            Msb = pool.tile([6, 32], F32)
            V.tensor_copy(Msb[:, :], psMom[:, :])

            # ---------------- preconditioned normal equations ----------------
            # q-blocks kept separate; per-batch 1/s, 1/s^2 applied afterwards
            # on partitions (batch = psA partition), off the radius chain.
            psAX = psp.tile([8, 146], F32)
            psA0 = psAX[:, 0:73]
            psA3 = psAX[:, 73:146]
            psA12 = psp.tile([8, 73], F32)
            T.matmul(psA0, Msb[0:6, 0:8], EQG_t[:, 0:73],
                     start=True, stop=True)
            A0sb = pool.tile([8, 73], F32)
            V.tensor_copy(A0sb[:, :], psA0)
            for q in (1, 2):
                T.matmul(psA12[:, :], Msb[0:6, 8 * q:8 * q + 8],
                         EQG_t[:, 73 * q:73 * q + 73], start=(q == 1),
                         stop=(q == 2))
            T.matmul(psA3, Msb[0:6, 24:32], EQG_t[:, 219:292],
                     start=True, stop=True)

            # ---------------- per-batch scalars to partitions --------------
            V.tensor_scalar(out=PR[:, 0:8], in0=psPR[:, 0:8], scalar1=-1.0,
                            op0=ALU.mult, scalar2=cc.gmx, op1=ALU.add)
            V.tensor_scalar(out=PR[:, 32:40], in0=psPR[:, 64:72], scalar1=-1.0,
                            op0=ALU.mult, scalar2=cc.gmy, op1=ALU.add)
            V.tensor_copy(PR[:, 64:72], sRow[:, :])
            T.transpose(psSC[:, :], PR[:, :], ONES1[0:1, 0:1])
            SCC = pool.tile([8, 4], F32)
            V.tensor_copy(SCC[:, 2:3], psSC[64:72, :])              # s_dst
            A.activation(SCC[:, 0:1], psSC[0:8, :], ACTF.Copy)      # mx
            A.activation(SCC[:, 1:2], psSC[32:40, :], ACTF.Copy)    # my

            # ---------------- Horner / Neumann solve ----------------
            IR8T = pool.tile([8, 2], F32)
            V.reciprocal(IR8T[:, 0:1], SCC[:, 2:3])
            V.tensor_tensor(out=IR8T[:, 1:2], in0=IR8T[:, 0:1],
                            in1=IR8T[:, 0:1], op=ALU.mult)
            U2 = pool.tile([8, 73], F32)
            V.scalar_tensor_tensor(out=U2[:, :], in0=psA12[:, :],
                                   scalar=IR8T[:, 0:1], in1=A0sb[:, :],
                                   op0=ALU.mult, op1=ALU.add)
            V.scalar_tensor_tensor(out=U2[:, :], in0=psA3,
                                   scalar=IR8T[:, 1:2], in1=U2[:, :],
                                   op0=ALU.mult, op1=ALU.add)
            GT = pool.tile([8, 64], F32)
            V.tensor_tensor(
                out=GT[:, :].rearrange("p (i j) -> p i j", i=8, j=8),
                in0=IME_t[:, :].rearrange("p (i j) -> p i j", i=8, j=8),
                in1=U2[:, 0:72].rearrange("p (i k) -> p i k", i=8, k=9)[:, :, 0:8],
                op=ALU.subtract)
            C0 = pool.tile([8, 8], F32)
            V.tensor_copy(C0[:, :], U2[:, 8:72:9])
            Gv3 = GT[:, :].rearrange("p (i j) -> p i j", i=8, j=8)
            PH = pool.tile([8, 64], F32)
            PHv = PH[:, :].rearrange("p (i j) -> p i j", i=8, j=8)
            YT = pool.tile([8, 8], F32)
            XC = pool.tile([8, 8], F32)
            for it in range(KHORNER):
                xin = C0 if it == 0 else XC
                V.tensor_tensor(out=PHv, in0=Gv3,
                                in1=xin[:, :].unsqueeze(1).broadcast_to([8, 8, 8]),
                                op=ALU.mult)
                V.tensor_reduce(out=YT[:, :], in_=PHv,
                                axis=mybir.AxisListType.X, op=ALU.add)
                xout = HN[:, 0:8] if it == KHORNER - 1 else XC[:, :]
                V.tensor_tensor(out=xout, in0=YT[:, :], in1=C0[:, :], op=ALU.add)

            # ---------------- denormalize + gate ----------------
            V.tensor_scalar(out=SCC[:, 3:4], in0=U2[:, 72:73],
                            scalar1=NPTS * 1e-4, op0=ALU.is_gt, scalar2=None)
            IG = pool.tile([8, 1], F32)
            V.tensor_scalar(out=IG[:, :], in0=SCC[:, 3:4], scalar1=-1.0,
                            op0=ALU.mult, scalar2=1.0, op1=ALU.add)
            TI = pool.tile([8, 9], F32)
            V.tensor_scalar(out=TI[:, :], in0=IEYE[:, :], scalar1=IG[:, :],
                            op0=ALU.mult, scalar2=None)
            mx_sc, my_sc = SCC[:, 0:1], SCC[:, 1:2]
            s_sc, g_sc = SCC[:, 2:3], SCC[:, 3:4]
            H2 = pool.tile([8, 9], F32)
            # H2[2,2] = c_ts*h6 + d_ts*h7 + 1 (early, feeds the sign chain)
            W1 = pool.tile([8, 1], F32)
            V.tensor_scalar(out=W1[:, :], in0=HN[:, 6:7], scalar1=cc.c_ts,
                            op0=ALU.mult, scalar2=1.0, op1=ALU.add)
            V.scalar_tensor_tensor(out=H2[:, 8:9], in0=HN[:, 7:8], scalar=cc.d_ts,
                                   in1=W1[:, :], op0=ALU.mult, op1=ALU.add)
            ABSD = pool.tile([8, 1], F32)
            NEGH = pool.tile([8, 1], F32)
            V.tensor_scalar(out=NEGH[:, :], in0=H2[:, 8:9], scalar1=-1.0,
                            op0=ALU.mult, scalar2=None)
            V.tensor_tensor(out=ABSD[:, :], in0=H2[:, 8:9], in1=NEGH[:, :],
                            op=ALU.max)
            SGN = pool.tile([8, 1], F32)
            V.tensor_scalar(out=SGN[:, :], in0=H2[:, 8:9], scalar1=0.0,
                            op0=ALU.is_lt, scalar2=-2.0, op1=ALU.mult)
            V.tensor_scalar(out=SGN[:, :], in0=SGN[:, :], scalar1=1.0,
                            op0=ALU.add, scalar2=None)
            DEN = pool.tile([8, 1], F32)
            V.tensor_scalar(out=DEN[:, :], in0=ABSD[:, :], scalar1=1e-8,
                            op0=ALU.max, scalar2=SGN[:, :], op1=ALU.mult)
            RECD = pool.tile([8, 1], F32)
            V.reciprocal(RECD[:, :], DEN[:, :])
            RG = pool.tile([8, 1], F32)
            V.tensor_tensor(out=RG[:, :], in0=RECD[:, :], in1=g_sc, op=ALU.mult)
            # rows of inv(T_dst) @ Hn
            T1 = pool.tile([8, 6], F32)
            H1 = pool.tile([8, 9], F32)
            V.tensor_scalar(out=T1[:, :], in0=HN[:, 0:6], scalar1=s_sc,
                            op0=ALU.mult, scalar2=None)
            V.scalar_tensor_tensor(out=H1[:, 0:3], in0=HN[:, 6:9], scalar=mx_sc,
                                   in1=T1[:, 0:3], op0=ALU.mult, op1=ALU.add)
            V.scalar_tensor_tensor(out=H1[:, 3:6], in0=HN[:, 6:9], scalar=my_sc,
                                   in1=T1[:, 3:6], op0=ALU.mult, op1=ALU.add)
            V.tensor_copy(H1[:, 6:9], HN[:, 6:9])
            # columns: @ T_src
            H1v = H1[:, :].rearrange("p (r c) -> p r c", r=3, c=3)
            H2v = H2[:, :].rearrange("p (r c) -> p r c", r=3, c=3)
            V.tensor_scalar(out=H2v[:, :, 0:2], in0=H1v[:, :, 0:2],
                            scalar1=cc.a_ts, op0=ALU.mult, scalar2=None)
            T2 = pool.tile([8, 2], F32)
            T3 = pool.tile([8, 2], F32)
            V.tensor_scalar(out=T2[:, :], in0=H1[:, 0:4:3], scalar1=cc.c_ts,
                            op0=ALU.mult, scalar2=None)
            V.scalar_tensor_tensor(out=T3[:, :], in0=H1[:, 1:5:3], scalar=cc.d_ts,
                                   in1=T2[:, :], op0=ALU.mult, op1=ALU.add)
            V.tensor_tensor(out=H2[:, 2:6:3], in0=T3[:, :], in1=H1[:, 2:6:3],
                            op=ALU.add)
            OUTt = pool.tile([8, 9], F32)
            V.scalar_tensor_tensor(out=OUTt[:, :], in0=H2[:, :], scalar=RG[:, :],
                                   in1=TI[:, :], op0=ALU.mult, op1=ALU.add)
            S.dma_start(Hout.ap().rearrange("b r c -> b (r c)"), OUTt[:, :])

    nc.compile()
    return nc


# ---------------------------------------------------------------------------
# host wrapper
# ---------------------------------------------------------------------------

_CACHE = {}


def _get(img_h, img_w):
    key = (int(img_h), int(img_w))
    if key not in _CACHE:
        cc = _Consts(*key)
        _CACHE[key] = (cc, _build_program(cc))
    return _CACHE[key]


def _in_maps(cc, flow, mask):
    flow = np.ascontiguousarray(flow, np.float32)
    mask = np.ascontiguousarray(mask, np.float32)
    rows2 = np.stack([cc.y0, cc.y0 + 1], -1).reshape(-1)   # (64,) = (k, a)
    cols2 = np.stack([cc.x0, cc.x0 + 1], -1).reshape(-1)   # (64,) = (i, d)
    maps = []
    for cix in range(NCORES):
        fl = flow[cix * BPC:(cix + 1) * BPC]          # (8, 2, H, W)
        mk = mask[cix * BPC:(cix + 1) * BPC]          # (8, 1, H, W)
        # FR [p=(c,kd,b), f=(sg,a,d,i)]
        fr = fl[:, :, rows2][:, :, :, cols2]          # (8, 2, 64, 64)
        fr = fr.reshape(BPC, 2, 8, 4, 2, 32, 2)       # b c kd sg a i d
        FR = fr.transpose(1, 2, 0, 3, 4, 6, 5).reshape(128, 512)
        # MR [p=(a,kd,b), f=(sg,d,i)]
        mr = mk[:, 0][:, rows2][:, :, cols2]          # (8, 64, 64)
        mr = mr.reshape(BPC, 8, 4, 2, 32, 2)          # b kd sg a i d
        MR = mr.transpose(3, 1, 0, 2, 5, 4).reshape(128, 256)
        fmr = np.ascontiguousarray(
            np.concatenate([FR, MR], axis=1).astype(ml_dtypes.bfloat16))
        maps.append({"FMR": fmr, "CALL1": cc.CALL1, "CALL2": cc.CALL2,
                     "CBF": cc.CBF})
    return maps


def run(flow, mask, img_h, img_w, trace=False, **spmd_kwargs):
    cc, nc = _get(img_h, img_w)
    res = bass_utils.run_bass_kernel_spmd(
        nc, _in_maps(cc, flow, mask), list(range(NCORES)), trace=trace, **spmd_kwargs
    )
    out = np.concatenate([res.results[c]["H"] for c in range(NCORES)], axis=0)
    return out.astype(np.float32), res


def kernel(flow, mask, img_h, img_w):
    out, _ = run(flow, mask, img_h, img_w)
    return out


# revision 45
# speedup vs baseline: 1.0298x; 1.0007x over previous
"""Trainium2 Bass kernel for DifferentiableDLT (batched weighted-DLT homography fit).

Contract: kernel(**inputs) takes FULL inputs
    flow (64, 2, 320, 576) f32, mask (64, 1, 320, 576) f32, img_h, img_w
and returns the FULL output (64, 3, 3) f32.

Design (pure data parallel, 8 batches/core x 8 cores). The 1024 sample points
form a fixed separable 32x32 grid; bilinear sampling touches only a 64-row x
64-column cross product of each field.
  Host (layout only, no arithmetic): shards the batch and gathers exactly the
  needed 64x64 footprint per (image, channel) into FMR [128, 768] bf16 in
  SBUF-ready partition order (flow p=(c,kd,b) free=(sg,a,d,i); mask
  p=(a,kd,b) free=(sg,d,i)), plus three packed constant blobs.
  Device, per core:
    1. Four DMAs split over the two HWDGE rings (S: FMR + small-partition
       consts; A: bf16 interp weights + 128-wide consts); everything lands
       within ~2.5us. Scalar-engine act tables prewarmed meanwhile.
    2. Bilinear interp = one [128,512] DVE multiply against a 4-tap weight
       table (image-coord scale folded in) + two tree-adds -> samples with
       points in the free dim.
    3. PE identity-matmul transpose + grid-offset matmul -> dst image coords
       in PSUM, points on partitions.
    4. Hartley stats via row-sums + ones-matmul broadcast (means) and
       sqrt + ones-matmul (radius); weighted features D = [w, w*cx, w*cy,
       w*r2]; moments C^T @ D on the PE.
    5. Normal equations assembled by a PE matmul against EQG = M0inv @ E
       (M0 = host-constant ideal normal matrix), directly yielding the
       preconditioned system G = I - M0inv*A, c0 = M0inv*b, and the support
       sum. Solved with 4 Horner steps of the Neumann series (spectral
       radius ~0.09); an exact-to-fp32-noise solve at ~1/3 the Gauss-Jordan
       latency.
    6. Denormalize H = inv(T_dst) Hn T_src, sign/scale fix, support gate,
       DMA out (8,3,3).
"""

import math
import numpy as np

import concourse.bass as bass
import concourse.bacc as bacc
import concourse.mybir as mybir
import ml_dtypes
from concourse import tile
from concourse import bass_utils

F32 = mybir.dt.float32
BF16 = mybir.dt.bfloat16
ALU = mybir.AluOpType
ACTF = mybir.ActivationFunctionType

NCORES = 8
BPC = 8          # batches per core
HF, WF = 320, 576
NG = 32          # grid is NG x NG points
NPTS = NG * NG
EPS = 1e-6
KHORNER = 4      # Neumann/Horner applications

# ---------------------------------------------------------------------------
# host-side constant computation
# ---------------------------------------------------------------------------


def _grid_1d(size, n):
    m = int(size * 0.05)
    return np.linspace(m, size - m - 1, n, dtype=np.float32)


class _Consts:
    def __init__(self, img_h, img_w):
        ys = _grid_1d(HF, NG)
        xs = _grid_1d(WF, NG)
        y0 = np.floor(ys).astype(np.int64)
        x0 = np.floor(xs).astype(np.int64)
        wy = (ys - y0).astype(np.float64)
        wx = (xs - x0).astype(np.float64)
        sx = np.float64(np.float32((img_w - 1) / max(WF - 1, 1)))
        sy = np.float64(np.float32((img_h - 1) / max(HF - 1, 1)))
        self.y0 = y0
        self.x0 = x0

        # ---- 4-tap interp weights ----
        wya = np.stack([1 - wy, wy], -1)        # (32, 2) [k, a]
        wxc = np.stack([1 - wx, wx], -1)        # (32, 2) [i, d]
        p = np.arange(128)
        # W4F [p=(c,kd,b), f=(sg,a,d,i)] = wya[4kd+sg, a]*wxc[i, d]*scale(c)
        kd_f = (p % 64) // 8
        c_f = p // 64
        kk = 4 * kd_f[:, None] + np.arange(4)[None, :]          # (128, 4)
        wya_k = wya[kk]                                          # (128, 4, 2)
        sxy_f = np.where(c_f == 0, sx, sy)
        W4F = (wya_k[:, :, :, None, None] * wxc.T[None, None, None, :, :]
               * sxy_f[:, None, None, None, None]).reshape(128, 512)
        # W4M [p=(a,kd,b), f=(sg,d,i)] = wya[4kd+sg, a_m]*wxc[i, d]
        a_m = p // 64
        kd_m = (p % 64) // 8
        kkm = 4 * kd_m[:, None] + np.arange(4)[None, :]
        wya_m = wya[kkm]                                         # (128, 4, 2)
        wya_sel = wya_m[np.arange(128)[:, None], np.arange(4)[None, :], a_m[:, None]]
        W4M = (wya_sel[:, :, None, None] * wxc.T[None, None, :, :]
               ).reshape(128, 256)

        # ---- means route: psPR = RSUM^T @ CBN128 = -flow-mean(c,b) (t-exp);
        # GRIDC[p=(sg,i), f=(c,kd,b)] = grid(c) - gridmean(c) in f32 ----
        j2 = np.arange(128)
        c_j2 = j2 // 64
        b_j2 = j2 % 8
        CBN128 = -(((c_f[:, None] == c_j2[None, :])
                    & ((p % 8)[:, None] == b_j2[None, :]))
                   .astype(np.float32) / NPTS)
        gmx = xs.astype(np.float64).mean() * sx
        gmy = ys.astype(np.float64).mean() * sy
        self.gmx = float(np.float32(gmx))
        self.gmy = float(np.float32(gmy))
        sg_p = p // 32
        i_p = p % 32
        kd_j2 = (j2 % 64) // 8
        gx_p = xs.astype(np.float64)[i_p] * sx - gmx            # (128,) by p
        gy_pf = (ys.astype(np.float64)[(4 * kd_j2[None, :] + sg_p[:, None])
                                       % 32] * sy - gmy)        # (128, 128)
        GRIDC = np.where(c_j2[None, :] == 0, gx_p[:, None], gy_pf
                         ).astype(np.float32)

        # ---- source-point features + T_src immediates ----
        jpt = np.arange(NPTS) // NG
        ipt = np.arange(NPTS) % NG
        gx = xs.astype(np.float64)[ipt]
        gy = ys.astype(np.float64)[jpt]
        sxi = gx * sx
        syi = gy * sy
        mx0, my0 = sxi.mean(), syi.mean()
        cxs, cys = sxi - mx0, syi - my0
        s_src = max(np.sqrt(cxs * cxs + cys * cys).mean() / math.sqrt(2.0), 1e-8)
        u = cxs / s_src
        v = cys / s_src
        self.a_ts = float(np.float32(1.0 / s_src))
        self.c_ts = float(np.float32(-mx0 / s_src))
        self.d_ts = float(np.float32(-my0 / s_src))
        feats = np.stack([u * u, u * v, u, v * v, v, np.ones_like(u)], -1)
        C6 = np.ascontiguousarray(
            feats.reshape(8, 128, 6).transpose(1, 0, 2).reshape(128, 48)
        ).astype(np.float32)

        # ---- E matrices: AUG[r*9+c] = sum_q sum_m E[q][m, r*9+c] * Mq[m] ----
        E = np.zeros((4, 6, 72))
        sym = [[0, 1, 2], [1, 3, 4], [2, 4, 5]]
        for r in range(3):
            for c in range(3):
                m = sym[r][c]
                E[0, m, r * 9 + c] += 1
                E[0, m, (r + 3) * 9 + (c + 3)] += 1
        cr = [[0, 1], [1, 3], [2, 4]]
        for q, r0 in ((1, 0), (2, 3)):
            for r in range(3):
                for c2 in range(2):
                    m = cr[r][c2]
                    E[q, m, (r0 + r) * 9 + 6 + c2] += -1
                    E[q, m, (6 + c2) * 9 + (r0 + r)] += -1
            for r, m in ((0, 2), (1, 4), (2, 5)):
                E[q, m, (r0 + r) * 9 + 8] += 1
        rb = [[0, 1], [1, 3]]
        for r in range(2):
            for c2 in range(2):
                E[3, rb[r][c2], (6 + r) * 9 + 6 + c2] += 1
        E[3, 2, 6 * 9 + 8] += -1
        E[3, 4, 7 * 9 + 8] += -1

        # ---- M0 (ideal normal matrix) -> EQG = M0inv @ E, IME = I-eps*M0inv
        o = np.ones_like(u)
        z = np.zeros_like(u)
        r1 = np.stack([u, v, o, z, z, z, -u * u, -u * v], -1)
        r2 = np.stack([z, z, z, u, v, o, -v * u, -v * v], -1)
        A0 = np.concatenate([r1, r2], 0) * math.sqrt(0.5)
        M0 = A0.T @ A0 + EPS * np.eye(8)
        M0inv = np.linalg.inv(M0)
        EQG = np.einsum('ir,qmrc->qmic', M0inv,
                        E.reshape(4, 6, 8, 9)).reshape(4, 6, 72)
        EQG73 = np.zeros((4, 6, 73))
        EQG73[:, :, 0:72] = EQG
        EQG73[0, 5, 72] = 1.0  # col 72 of q=0 block picks S1 = sum(w)
        EQGP = np.ascontiguousarray(
            EQG73.transpose(1, 0, 2).reshape(6, 292)).astype(np.float32)
        IME = np.tile((np.eye(8) - EPS * M0inv).reshape(1, 64),
                      (8, 1)).astype(np.float32)

        # ---- packed constant blobs ----
        # CALL1 [128, 307] f32: C6 | CBN128 | GRIDC | ONESC | ONES1 | ONESROW
        IDN = np.eye(128, dtype=np.float32)  # shipped bf16 in CBF
        c1 = np.zeros((128, 307), np.float32)
        c1[:, 0:48] = C6
        c1[:, 48:176] = CBN128
        c1[:, 176:304] = GRIDC
        c1[:, 304] = 1.0 / (1024 * math.sqrt(2.0))
        c1[:, 305] = 1.0
        c1[0, 306] = 1.0  # ONESROW lives on p0 via transpose? no: see CALL2
        self.CALL1 = c1
        # CALL2 [8, 512] f32: EQG c0:292 | IME c292:356 | ONESROW c356:484
        #                    | IEYE c484:493 | HNINIT c493:502 | ACTJ c502:504
        #                    | PRZ c504:512 (row 0 only for ONESROW/PRZ)
        c2b = np.zeros((8, 512), np.float32)
        c2b[0:6, 0:292] = EQGP
        c2b[0:8, 292:356] = IME
        c2b[0, 356:484] = 1.0
        ie = np.zeros((3, 3), np.float32); np.fill_diagonal(ie, 1.0)
        c2b[0:8, 484:493] = ie.reshape(1, 9)
        c2b[0:8, 501] = 1.0   # HN col 8 = 1.0 at c493+8
        c2b[0:8, 502:504] = 1.0
        self.CALL2 = c2b
        # CBF [128, 896] bf16: W4F | W4M | IDN
        cb = np.zeros((128, 896), np.float32)
        cb[:, 0:512] = W4F
        cb[:, 512:768] = W4M
        cb[:, 768:896] = IDN
        self.CBF = cb.astype(ml_dtypes.bfloat16)


# ---------------------------------------------------------------------------
# device program
# ---------------------------------------------------------------------------


def _build_program(cc: _Consts):
    nc = bacc.Bacc("TRN2", target_bir_lowering=False, debug=False,
                   num_swdge_queues=1)

    FMR = nc.dram_tensor("FMR", [128, 768], BF16, kind="ExternalInput")
    CALL1 = nc.dram_tensor("CALL1", [128, 307], F32, kind="ExternalInput")
    CALL2 = nc.dram_tensor("CALL2", [8, 512], F32, kind="ExternalInput")
    CBF = nc.dram_tensor("CBF", [128, 896], BF16, kind="ExternalInput")
    Hout = nc.dram_tensor("H", [BPC, 3, 3], F32, kind="ExternalOutput")

    V = nc.vector
    A = nc.scalar
    T = nc.tensor
    S = nc.sync

    with tile.TileContext(nc) as tc:
        with (
            tc.tile_pool(name="sb", bufs=1) as pool,
            tc.tile_pool(name="ps", bufs=1, space="PSUM") as psp,
        ):
            # ---------------- DMAs: critical blobs first on each ring -------
            # flow data (S ring) and flow weights (A ring) land first so the
            # interp multiply can start ASAP; mask halves next; consts last.
            FMR_t = pool.tile([128, 768], BF16, tag="FMR")
            CALL1_t = pool.tile([128, 307], F32, tag="CALL1")
            CALL2_t = pool.tile([8, 512], F32, tag="CALL2")
            CBF_t = pool.tile([128, 896], BF16, tag="CBF")
            A.dma_start(CBF_t[:, 0:512], CBF.ap()[:, 0:512])
            S.dma_start(FMR_t[:, 0:512], FMR.ap()[:, 0:512])
            A.dma_start(CBF_t[:, 512:896], CBF.ap()[:, 512:896])
            S.dma_start(FMR_t[:, 512:768], FMR.ap()[:, 512:768])
            A.dma_start(CALL1_t[:, :], CALL1.ap())
            S.dma_start(CALL2_t[:, :], CALL2.ap())

            C6_t = CALL1_t[:, 0:48]
            CBN_t = CALL1_t[:, 48:176]
            GRIDC_t = CALL1_t[:, 176:304]
            EQG_t = CALL2_t[0:6, 0:292]
            IME_t = CALL2_t[0:8, 292:356]
            W4F_t = CBF_t[:, 0:512]
            W4M_t = CBF_t[:, 512:768]
            IDNB_t = CBF_t[:, 768:896]

            # ---------------- DMA-shipped consts + ACT warmups -------------
            ONESC = CALL1_t[:, 304:305]
            ONES1 = CALL1_t[:, 305:306]
            ONESROW = CALL2_t[0:1, 356:484]
            IEYE = CALL2_t[0:8, 484:493]
            HN = pool.tile([8, 9], F32)
            V.tensor_copy(HN[:, 8:9], CALL2_t[0:8, 501:502])
            PR = pool.tile([1, 128], F32)
            V.memset(PR[:, :], 0.0)
            ACTJ = pool.tile([8, 2], F32)
            A.activation(ACTJ[:, 0:1], CALL2_t[0:8, 502:503], ACTF.Sqrt)
            A.activation(ACTJ[:, 0:1], CALL2_t[0:8, 503:504], ACTF.Copy)

            psF = psp.tile([128, 128], F32)
            psM = psp.tile([128, 64], F32)
            psSCW = psp.tile([128, 2], F32)
            psSC = psSCW[:, 0:1]
            D = pool.tile([128, 256], F32)
            Dv = D[:, :].rearrange("p (t q b) -> p t q b", t=8, q=4, b=8)

            # ---------------- flow: interp + transpose ---------------------
            # high priority: keep this chain contiguous at the head of the
            # Vector stream so the mask ops (whose DMA lands later) cannot
            # stall it.
            P = pool.tile([128, 512], BF16)
            Pv = P[:, :].rearrange("p (s a d i) -> p s a d i",
                                   s=4, a=2, d=2, i=32)
            tFv = FMR_t[:, 0:512].rearrange("p (s a d i) -> p s a d i",
                                            s=4, a=2, d=2, i=32)
            W4v = W4F_t.rearrange("p (s a d i) -> p s a d i",
                                  s=4, a=2, d=2, i=32)
            Q = pool.tile([128, 256], BF16)
            Qv = Q[:, :].rearrange("p (s d i) -> p s d i", s=4, d=2, i=32)
            samp = pool.tile([128, 128], BF16)
            sampv = samp[:, :].rearrange("p (s i) -> p s i", s=4, i=32)
            RSUM = pool.tile([128, 1], F32)
            with tc.high_priority():
                # two sg-halves pipelined against the chunked DMAs
                V.tensor_tensor(out=Pv[:, 0:2], in0=tFv[:, 0:2],
                                in1=W4v[:, 0:2], op=ALU.mult)
                V.tensor_tensor(out=Qv[:, 0:2], in0=Pv[:, 0:2, 0, :, :],
                                in1=Pv[:, 0:2, 1, :, :], op=ALU.add)
                V.tensor_tensor(out=sampv[:, 0:2], in0=Qv[:, 0:2, 0, :],
                                in1=Qv[:, 0:2, 1, :], op=ALU.add)
                V.tensor_tensor(out=Pv[:, 2:4], in0=tFv[:, 2:4],
                                in1=W4v[:, 2:4], op=ALU.mult)
                V.tensor_tensor(out=Qv[:, 2:4], in0=Pv[:, 2:4, 0, :, :],
                                in1=Pv[:, 2:4, 1, :, :], op=ALU.add)
                V.tensor_tensor(out=sampv[:, 2:4], in0=Qv[:, 2:4, 0, :],
                                in1=Qv[:, 2:4, 1, :], op=ALU.add)
                V.tensor_reduce(out=RSUM[:, :], in_=samp[:, :].unsqueeze(1),
                                axis=mybir.AxisListType.X, op=ALU.add)
            # psPR[0, f=(c,t,b)] = -flow-mean(c,b); broadcast to all point
            # partitions via a rank-1 ones matmul accumulated into psF.
            T.matmul(psF[0:64, :], samp[:, 0:64], IDNB_t,
                     start=True, stop=False)
            T.matmul(psF[64:128, :], samp[:, 64:128], IDNB_t,
                     start=True, stop=False)
            psPR = psp.tile([1, 128], F32)
            T.matmul(psPR[:, :], RSUM[:, :], CBN_t, start=True, stop=True)
            PRN = pool.tile([1, 128], F32)
            V.tensor_copy(PRN[:, :], psPR[:, :])
            T.matmul(psF[0:64, :], ONESROW[:, 0:64], PRN[:, :],
                     start=False, stop=True)
            T.matmul(psF[64:128, :], ONESROW[:, 0:64], PRN[:, :],
                     start=False, stop=True)

            # ---------------- mask: interp + transpose + relu --------------
            PM = pool.tile([128, 256], BF16)
            V.tensor_tensor(out=PM[:, :], in0=FMR_t[:, 512:768], in1=W4M_t,
                            op=ALU.mult)
            PMhi = pool.tile([64, 256], BF16)
            V.tensor_copy(PMhi[:, :], PM[64:128, :])
            SM = pool.tile([64, 256], BF16)
            V.tensor_tensor(out=SM[:, :], in0=PM[0:64, :], in1=PMhi[:, :],
                            op=ALU.add)
            sampM = pool.tile([64, 128], BF16)
            SMv = SM[:, :].rearrange("p (s d i) -> p s d i", s=4, d=2, i=32)
            smv = sampM[:, :].rearrange("p (s i) -> p s i", s=4, i=32)
            V.tensor_tensor(out=smv, in0=SMv[:, :, 0, :], in1=SMv[:, :, 1, :],
                            op=ALU.add)
            T.matmul(psM[:, :], sampM[:, :], IDNB_t[0:64, 0:64],
                     start=True, stop=True)
            V.tensor_scalar(
                out=Dv[:, :, 0, :],
                in0=psM[:, :].rearrange("p (t b) -> p t b", t=8, b=8),
                scalar1=0.0, op0=ALU.max, scalar2=None)

            # ---------------- radius (CXY = centered dst coords) -----------
            CXY = pool.tile([128, 128], F32)
            V.tensor_tensor(out=CXY[:, :], in0=psF[:, :], in1=GRIDC_t,
                            op=ALU.add)
            SQ = pool.tile([128, 128], F32, tag="SQ")
            V.tensor_tensor(out=SQ[:, :], in0=CXY[:, :], in1=CXY[:, :],
                            op=ALU.mult)
            R2 = pool.tile([128, 64], F32)     # [pl, (t, b)]
            V.tensor_tensor(out=R2[:, :], in0=SQ[:, 0:64], in1=SQ[:, 64:128],
                            op=ALU.add)
            SQR = pool.tile([128, 64], F32)
            A.activation(SQR[:, :], R2[:, :], ACTF.Sqrt)
            psSq = psp.tile([1, 64], F32)
            with tc.high_priority():
                T.matmul(psSq[:, :], ONESC[:, :], SQR[:, :],
                         start=True, stop=True)
            sRow = pool.tile([1, 8], F32)
            V.tensor_reduce(
                out=sRow[:, :],
                in_=psSq[:, :].rearrange("o (t b) -> o b t", t=8, b=8),
                axis=mybir.AxisListType.X, op=ALU.add)
            V.tensor_scalar(out=sRow[:, :], in0=sRow[:, :],
                            scalar1=1e-8, op0=ALU.max, scalar2=None)

            # ---------------- D features + moments ----------------
            V.tensor_tensor(
                out=Dv[:, :, 1:3, :],
                in0=CXY[:, :].rearrange("p (c t b) -> p t c b", c=2, t=8, b=8),
                in1=Dv[:, :, 0:1, :].broadcast_to([128, 8, 2, 8]), op=ALU.mult)
            V.tensor_tensor(
                out=Dv[:, :, 3, :],
                in0=R2[:, :].rearrange("p (t b) -> p t b", t=8, b=8),
                in1=Dv[:, :, 0, :], op=ALU.mult)
            psMom = psp.tile([6, 32], F32)
            for t in range(8):
                T.matmul(psMom[:, :], C6_t[:, 6 * t:6 * t + 6],
                         D[:, 32 * t:32 * t + 32], start=(t == 0), stop=(t == 7))
            Msb = pool.tile([6, 32], F32)
            V.tensor_copy(Msb[:, :], psMom[:, :])

            # ---------------- preconditioned normal equations ----------------
            # q-blocks kept separate; per-batch 1/s, 1/s^2 applied afterwards
            # on partitions (batch = psA partition), off the radius chain.
            psAX = psp.tile([8, 146], F32)
            psA0 = psAX[:, 0:73]
            psA3 = psAX[:, 73:146]
            psA12 = psp.tile([8, 73], F32)
            T.matmul(psA0, Msb[0:6, 0:8], EQG_t[:, 0:73],
                     start=True, stop=True)
            A0sb = pool.tile([8, 73], F32)
            V.tensor_copy(A0sb[:, :], psA0)
            for q in (1, 2):
                T.matmul(psA12[:, :], Msb[0:6, 8 * q:8 * q + 8],
                         EQG_t[:, 73 * q:73 * q + 73], start=(q == 1),
                         stop=(q == 2))
            T.matmul(psA3, Msb[0:6, 24:32], EQG_t[:, 219:292],
                     start=True, stop=True)

            # ---------------- per-batch scalars to partitions --------------
            V.tensor_scalar(out=PR[:, 0:8], in0=psPR[:, 0:8], scalar1=-1.0,
                            op0=ALU.mult, scalar2=cc.gmx, op1=ALU.add)
            V.tensor_scalar(out=PR[:, 32:40], in0=psPR[:, 64:72], scalar1=-1.0,
                            op0=ALU.mult, scalar2=cc.gmy, op1=ALU.add)
            V.tensor_copy(PR[:, 64:72], sRow[:, :])
            T.transpose(psSC[:, :], PR[:, :], ONES1[0:1, 0:1])
            SCC = pool.tile([8, 4], F32)
            V.tensor_copy(SCC[:, 2:3], psSC[64:72, :])              # s_dst
            A.activation(SCC[:, 0:1], psSC[0:8, :], ACTF.Copy)      # mx
            A.activation(SCC[:, 1:2], psSC[32:40, :], ACTF.Copy)    # my

            # ---------------- Horner / Neumann solve ----------------
            IR8T = pool.tile([8, 2], F32)
            V.reciprocal(IR8T[:, 0:1], SCC[:, 2:3])
            V.tensor_tensor(out=IR8T[:, 1:2], in0=IR8T[:, 0:1],
                            in1=IR8T[:, 0:1], op=ALU.mult)
            U2 = pool.tile([8, 73], F32)
            V.scalar_tensor_tensor(out=U2[:, :], in0=psA12[:, :],
                                   scalar=IR8T[:, 0:1], in1=A0sb[:, :],
                                   op0=ALU.mult, op1=ALU.add)
            V.scalar_tensor_tensor(out=U2[:, :], in0=psA3,
                                   scalar=IR8T[:, 1:2], in1=U2[:, :],
                                   op0=ALU.mult, op1=ALU.add)
            GT = pool.tile([8, 64], F32)
            V.tensor_tensor(
                out=GT[:, :].rearrange("p (i j) -> p i j", i=8, j=8),
                in0=IME_t[:, :].rearrange("p (i j) -> p i j", i=8, j=8),
                in1=U2[:, 0:72].rearrange("p (i k) -> p i k", i=8, k=9)[:, :, 0:8],
                op=ALU.subtract)
            C0 = pool.tile([8, 8], F32)
            V.tensor_copy(C0[:, :], U2[:, 8:72:9])
            Gv3 = GT[:, :].rearrange("p (i j) -> p i j", i=8, j=8)
            PH = pool.tile([8, 64], F32)
            PHv = PH[:, :].rearrange("p (i j) -> p i j", i=8, j=8)
            YT = pool.tile([8, 8], F32)
            XC = pool.tile([8, 8], F32)
            for it in range(KHORNER):
                xin = C0 if it == 0 else XC
                V.tensor_tensor(out=PHv, in0=Gv3,
                                in1=xin[:, :].unsqueeze(1).broadcast_to([8, 8, 8]),
                                op=ALU.mult)
                V.tensor_reduce(out=YT[:, :], in_=PHv,
                                axis=mybir.AxisListType.X, op=ALU.add)
                xout = HN[:, 0:8] if it == KHORNER - 1 else XC[:, :]
                V.tensor_tensor(out=xout, in0=YT[:, :], in1=C0[:, :], op=ALU.add)

            # ---------------- denormalize + gate ----------------
            V.tensor_scalar(out=SCC[:, 3:4], in0=U2[:, 72:73],
                            scalar1=NPTS * 1e-4, op0=ALU.is_gt, scalar2=None)
            IG = pool.tile([8, 1], F32)
            V.tensor_scalar(out=IG[:, :], in0=SCC[:, 3:4], scalar1=-1.0,
                            op0=ALU.mult, scalar2=1.0, op1=ALU.add)
            TI = pool.tile([8, 9], F32)
            V.tensor_scalar(out=TI[:, :], in0=IEYE[:, :], scalar1=IG[:, :],
                            op0=ALU.mult, scalar2=None)
            mx_sc, my_sc = SCC[:, 0:1], SCC[:, 1:2]
            s_sc, g_sc = SCC[:, 2:3], SCC[:, 3:4]
            H2 = pool.tile([8, 9], F32)
            # H2[2,2] = c_ts*h6 + d_ts*h7 + 1 (early, feeds the sign chain)
            W1 = pool.tile([8, 1], F32)
            V.tensor_scalar(out=W1[:, :], in0=HN[:, 6:7], scalar1=cc.c_ts,
                            op0=ALU.mult, scalar2=1.0, op1=ALU.add)
            V.scalar_tensor_tensor(out=H2[:, 8:9], in0=HN[:, 7:8], scalar=cc.d_ts,
                                   in1=W1[:, :], op0=ALU.mult, op1=ALU.add)
            ABSD = pool.tile([8, 1], F32)
            NEGH = pool.tile([8, 1], F32)
            V.tensor_scalar(out=NEGH[:, :], in0=H2[:, 8:9], scalar1=-1.0,
                            op0=ALU.mult, scalar2=None)
            V.tensor_tensor(out=ABSD[:, :], in0=H2[:, 8:9], in1=NEGH[:, :],
                            op=ALU.max)
            SGN = pool.tile([8, 1], F32)
            V.tensor_scalar(out=SGN[:, :], in0=H2[:, 8:9], scalar1=0.0,
                            op0=ALU.is_lt, scalar2=-2.0, op1=ALU.mult)
            V.tensor_scalar(out=SGN[:, :], in0=SGN[:, :], scalar1=1.0,
                            op0=ALU.add, scalar2=None)
            DEN = pool.tile([8, 1], F32)
            V.tensor_scalar(out=DEN[:, :], in0=ABSD[:, :], scalar1=1e-8,
                            op0=ALU.max, scalar2=SGN[:, :], op1=ALU.mult)
            RECD = pool.tile([8, 1], F32)
            V.reciprocal(RECD[:, :], DEN[:, :])
            RG = pool.tile([8, 1], F32)
            V.tensor_tensor(out=RG[:, :], in0=RECD[:, :], in1=g_sc, op=ALU.mult)
            # rows of inv(T_dst) @ Hn
            T1 = pool.tile([8, 6], F32)
            H1 = pool.tile([8, 9], F32)
            V.tensor_scalar(out=T1[:, :], in0=HN[:, 0:6], scalar1=s_sc,
                            op0=ALU.mult, scalar2=None)
            V.scalar_tensor_tensor(out=H1[:, 0:3], in0=HN[:, 6:9], scalar=mx_sc,
                                   in1=T1[:, 0:3], op0=ALU.mult, op1=ALU.add)
            V.scalar_tensor_tensor(out=H1[:, 3:6], in0=HN[:, 6:9], scalar=my_sc,
                                   in1=T1[:, 3:6], op0=ALU.mult, op1=ALU.add)
            V.tensor_copy(H1[:, 6:9], HN[:, 6:9])
            # columns: @ T_src
            H1v = H1[:, :].rearrange("p (r c) -> p r c", r=3, c=3)
            H2v = H2[:, :].rearrange("p (r c) -> p r c", r=3, c=3)
            V.tensor_scalar(out=H2v[:, :, 0:2], in0=H1v[:, :, 0:2],
                            scalar1=cc.a_ts, op0=ALU.mult, scalar2=None)
            T2 = pool.tile([8, 2], F32)
            T3 = pool.tile([8, 2], F32)
            V.tensor_scalar(out=T2[:, :], in0=H1[:, 0:4:3], scalar1=cc.c_ts,
                            op0=ALU.mult, scalar2=None)
            V.scalar_tensor_tensor(out=T3[:, :], in0=H1[:, 1:5:3], scalar=cc.d_ts,
                                   in1=T2[:, :], op0=ALU.mult, op1=ALU.add)
            V.tensor_tensor(out=H2[:, 2:6:3], in0=T3[:, :], in1=H1[:, 2:6:3],
                            op=ALU.add)
            OUTt = pool.tile([8, 9], F32)
            V.scalar_tensor_tensor(out=OUTt[:, :], in0=H2[:, :], scalar=RG[:, :],
                                   in1=TI[:, :], op0=ALU.mult, op1=ALU.add)
            S.dma_start(Hout.ap().rearrange("b r c -> b (r c)"), OUTt[:, :])

    nc.compile()
    return nc


# ---------------------------------------------------------------------------
# host wrapper
# ---------------------------------------------------------------------------

_CACHE = {}


def _get(img_h, img_w):
    key = (int(img_h), int(img_w))
    if key not in _CACHE:
        cc = _Consts(*key)
        _CACHE[key] = (cc, _build_program(cc))
    return _CACHE[key]


def _in_maps(cc, flow, mask):
    flow = np.ascontiguousarray(flow, np.float32)
    mask = np.ascontiguousarray(mask, np.float32)
    rows2 = np.stack([cc.y0, cc.y0 + 1], -1).reshape(-1)   # (64,) = (k, a)
    cols2 = np.stack([cc.x0, cc.x0 + 1], -1).reshape(-1)   # (64,) = (i, d)
    maps = []
    for cix in range(NCORES):
        fl = flow[cix * BPC:(cix + 1) * BPC]          # (8, 2, H, W)
        mk = mask[cix * BPC:(cix + 1) * BPC]          # (8, 1, H, W)
        # FR [p=(c,kd,b), f=(sg,a,d,i)]
        fr = fl[:, :, rows2][:, :, :, cols2]          # (8, 2, 64, 64)
        fr = fr.reshape(BPC, 2, 8, 4, 2, 32, 2)       # b c kd sg a i d
        FR = fr.transpose(1, 2, 0, 3, 4, 6, 5).reshape(128, 512)
        # MR [p=(a,kd,b), f=(sg,d,i)]
        mr = mk[:, 0][:, rows2][:, :, cols2]          # (8, 64, 64)
        mr = mr.reshape(BPC, 8, 4, 2, 32, 2)          # b kd sg a i d
        MR = mr.transpose(3, 1, 0, 2, 5, 4).reshape(128, 256)
        fmr = np.ascontiguousarray(
            np.concatenate([FR, MR], axis=1).astype(ml_dtypes.bfloat16))
        maps.append({"FMR": fmr, "CALL1": cc.CALL1, "CALL2": cc.CALL2,
                     "CBF": cc.CBF})
    return maps


def run(flow, mask, img_h, img_w, trace=False, **spmd_kwargs):
    cc, nc = _get(img_h, img_w)
    res = bass_utils.run_bass_kernel_spmd(
        nc, _in_maps(cc, flow, mask), list(range(NCORES)), trace=trace, **spmd_kwargs
    )
    out = np.concatenate([res.results[c]["H"] for c in range(NCORES)], axis=0)
    return out.astype(np.float32), res


def kernel(flow, mask, img_h, img_w):
    out, _ = run(flow, mask, img_h, img_w)
    return out
